# revision 2
# baseline (speedup 1.0000x reference)
"""Multi-head self-attention (RoPE, causal) Trainium2 Bass kernel.

Sharding: 8 cores = 2 batches x 4 head-groups (4 heads each).
Each core computes QKV projections for its heads (feature-major via x^T),
RoPE, causal attention with transposed scores (softmax along partitions
handled via exp + ones-column denominator in the V matmul), and a partial
output projection over its head slice. The host sums the 4 partials per
batch (reduce step of the tensor-parallel output projection).

All matmuls run in fp32r (fp32 data, high-half weights) at full PE rate.
"""
import math
import os
from contextlib import ExitStack

import numpy as np

import concourse.bass as bass
import concourse.tile as tile
from concourse import bacc, mybir

F32 = mybir.dt.float32
F32R = mybir.dt.float32r
EXP = mybir.ActivationFunctionType.Exp

B, S, D, H, DH = 2, 2048, 1024, 16, 64
THETA = 10000.0
CORES = 8
HPC = 4                    # heads per core
F = HPC * DH               # 256 features per core
SCALE = 1.0 / math.sqrt(DH)
NKT = D // 128             # 8 k tiles
NSB = S // 512             # 4 seq blocks of 512
NST = S // 128             # 16 seq tiles of 128

DEFAULT_OPTS = dict(norm="pe", reps=1, use_rs=False, rope_gp=False, v_first=False)

_CACHED = {}


def _split_load(nc, dst, src, chunk_cols=512):
    """DMA a [128, N] DRAM slab into SBUF as column chunks (queue parallelism)."""
    n = dst.shape[-1]
    for c0 in range(0, n, chunk_cols):
        c1 = min(n, c0 + chunk_cols)
        nc.sync.dma_start(out=dst[:, c0:c1], in_=src[:, c0:c1])


def _build_program(opts):
    use_rs = opts["use_rs"]
    reps = opts["reps"]
    nc = bacc.Bacc("TRN2", target_bir_lowering=False, debug=False,
                   num_devices=CORES)

    xT = nc.dram_tensor("xT", [D, S], F32R, kind="ExternalInput")
    wqT = nc.dram_tensor("wqT", [D, F], F32R, kind="ExternalInput")
    wkT = nc.dram_tensor("wkT", [D, F], F32R, kind="ExternalInput")
    wvT = nc.dram_tensor("wvT", [D, F], F32R, kind="ExternalInput")
    woT = nc.dram_tensor("woT", [F, D], F32R, kind="ExternalInput")
    ropeA_d = nc.dram_tensor("ropeA", [128, S], F32, kind="ExternalInput")
    ropeB2_d = nc.dram_tensor("ropeB2", [128, S], F32, kind="ExternalInput")
    tri_d = nc.dram_tensor("tri", [128, 128], F32R, kind="ExternalInput")

    if use_rs:
        out_d = nc.dram_tensor("out_rs", [S // 4, D], F32, kind="ExternalOutput")
        cc_ins = [nc.dram_tensor(f"cc_in{j}", [512, D], F32) for j in range(NSB)]
        cc_outs = [nc.dram_tensor(f"cc_out{j}", [128, D], F32) for j in range(NSB)]
        groups = [[0, 1, 2, 3], [4, 5, 6, 7]]
    else:
        out_d = nc.dram_tensor("partial", [S, D], F32, kind="ExternalOutput")

    den_dram = nc.dram_tensor("den_dram", [NSB * H // 2, 512], F32)  # scratch

    with tile.TileContext(nc) as tc, ExitStack() as ctx:
        persist = ctx.enter_context(tc.tile_pool(name="persist", bufs=1))

        # ---- persistent tiles ----
        wq_t = [persist.tile([128, F], F32R, tag=f"wq{k}", name=f"wq{k}") for k in range(NKT)]
        wk_t = [persist.tile([128, F], F32R, tag=f"wk{k}", name=f"wk{k}") for k in range(NKT)]
        wv_t = [persist.tile([128, F], F32R, tag=f"wv{k}", name=f"wv{k}") for k in range(NKT)]
        wo_t = [persist.tile([128, D], F32R, tag=f"wo{t}", name=f"wo{t}") for t in range(2)]
        ropeA = persist.tile([128, S], F32, tag="ropeA", name="ropeA")
        ropeB2 = persist.tile([128, S], F32, tag="ropeB2", name="ropeB2")
        tri = persist.tile([128, 128], F32R, tag="tri", name="tri")
        QT = [persist.tile([128, S], F32R, tag=f"QT{t}", name=f"QT{t}") for t in range(2)]
        KT = [persist.tile([128, S], F32R, tag=f"KT{t}", name=f"KT{t}") for t in range(2)]
        Vaug = [persist.tile([128, 260], F32R, tag=f"Vaug{st}", name=f"Vaug{st}") for st in range(NST)]
        attT = [persist.tile([128, S], F32R, tag=f"attT{t}", name=f"attT{t}") for t in range(2)]
        ones4 = persist.tile([128, 4], F32, tag="ones4", name="ones4")
        nc.vector.memset(ones4, 1.0)
        zeros384 = persist.tile([128, 384], F32, tag="zeros384", name="zeros384")
        nc.vector.memset(zeros384, 0.0)
        if opts["norm"] == "pe":
            ones1f = persist.tile([1, 64], F32, tag="ones1f", name="ones1f")
            nc.vector.memset(ones1f, 1.0)
            ones1 = persist.tile([1, 64], F32R, tag="ones1", name="ones1")
            nc.vector.tensor_copy(ones1, ones1f)
        xt = [persist.tile([128, S], F32R, tag=f"xt{k}", name=f"xt{k}")
              for k in range(NKT)]

        for k in range(NKT):
            nc.sync.dma_start(out=wq_t[k], in_=wqT[128 * k:128 * (k + 1), :])
            nc.sync.dma_start(out=wk_t[k], in_=wkT[128 * k:128 * (k + 1), :])
            _split_load(nc, xt[k], xT[128 * k:128 * (k + 1), :])
            nc.sync.dma_start(out=wv_t[k], in_=wvT[128 * k:128 * (k + 1), :])
        _split_load(nc, ropeA, ropeA_d)
        _split_load(nc, ropeB2, ropeB2_d)
        for t in range(2):
            _split_load(nc, wo_t[t], woT[128 * t:128 * (t + 1), :])
        nc.sync.dma_start(out=tri, in_=tri_d[:, :])

        for _rep in range(reps):
            _body(nc, tc, opts, locals())

    nc.compile()
    return nc


def _body(nc, tc, opts, env):
    use_rs = opts["use_rs"]
    xt = env["xt"]; wq_t = env["wq_t"]; wk_t = env["wk_t"]; wv_t = env["wv_t"]
    wo_t = env["wo_t"]; ropeA = env["ropeA"]; ropeB2 = env["ropeB2"]
    tri = env["tri"]; QT = env["QT"]; KT = env["KT"]; Vaug = env["Vaug"]
    attT = env["attT"]; ones4 = env["ones4"]; den_dram = env["den_dram"]
    zeros384 = env["zeros384"]
    out_d = env["out_d"]
    if use_rs:
        cc_ins = env["cc_ins"]; cc_outs = env["cc_outs"]; groups = env["groups"]
    if opts["norm"] == "pe":
        ones1 = env["ones1"]

    # ---- Phase 1: Q/K projections + RoPE (feature-major) ----
    with tc.tile_pool(name="ptmp", bufs=2) as ptmp, \
         tc.tile_pool(name="psProj", bufs=4, space="PSUM") as psProj, \
         tc.tile_pool(name="psV", bufs=2, space="PSUM") as psV:
        def _vpass():
            for st in range(NST):
                ss = slice(128 * st, 128 * (st + 1))
                ps = psV.tile([128, 256], F32, tag="projv", name="projv")
                for k in range(NKT):
                    nc.tensor.matmul(ps, xt[k][:, ss], wv_t[k],
                                     start=(k == 0), stop=(k == NKT - 1))
                dst = Vaug[st][:, 0:260].rearrange("p (h c) -> p h c", h=HPC)
                nc.scalar.copy(dst[:, :, 0:64],
                               ps.rearrange("p (h c) -> p h c", h=HPC))
                nc.vector.tensor_copy(Vaug[st][:, 64:260:65], ones4)

        if opts["v_first"]:
            _vpass()
        for sb in range(NSB):
            cs = slice(512 * sb, 512 * (sb + 1))
            for (w_t, dest) in ((wq_t, QT), (wk_t, KT)):
                for t in range(2):
                    fs = slice(128 * t, 128 * (t + 1))
                    ps = psProj.tile([128, 512], F32, tag="proj", name="proj")
                    for k in range(NKT):
                        nc.tensor.matmul(ps, w_t[k][:, fs], xt[k][:, cs],
                                         start=(k == 0), stop=(k == NKT - 1))
                    # rope: dest = raw*A + swap(raw*B2); psum->sbuf via idle ACT
                    raw = ptmp.tile([128, 512], F32, tag="raw", name="raw")
                    nc.scalar.copy(raw, ps)
                    t1 = ptmp.tile([128, 512], F32, tag="t1", name="t1")
                    nc.vector.tensor_mul(t1, raw, ropeA[:, cs])
                    t2p = ptmp.tile([128, 512], F32, tag="t2p", name="t2p")
                    nc.vector.tensor_mul(t2p, raw, ropeB2[:, cs])
                    t2 = ptmp.tile([128, 512], F32, tag="t2", name="t2")
                    for blk in range(4):
                        a, b = 32 * blk, 32 * (blk ^ 1)
                        nc.sync.dma_start(out=t2[a:a + 32, :],
                                          in_=t2p[b:b + 32, :])
                    if opts["rope_gp"]:
                        nc.gpsimd.tensor_add(dest[t][:, cs], t1, t2)
                    else:
                        nc.vector.tensor_add(dest[t][:, cs], t1, t2)

        # ---- Phase 2: V projection into Vaug (seq-major, ones cols) ----
        if not opts["v_first"]:
            _vpass()

    # ---- Phase 3+4: attention per seq block, then partial Wo ----
    with tc.tile_pool(name="psA", bufs=2, space="PSUM") as psA, \
         tc.tile_pool(name="psB", bufs=2, space="PSUM") as psB, \
         tc.tile_pool(name="epool", bufs=3) as epool, \
         tc.tile_pool(name="ntmp", bufs=2) as ntmp, \
         tc.tile_pool(name="opool", bufs=2) as opool:
        for j in range(NSB):
            qs = slice(512 * j, 512 * (j + 1))
            n_pair = 2 * (j + 1)
            for hp in range(2):
                t = hp
                pn = [psB.tile([65, 512], F32, tag="pn", name="pn") for _ in range(2)]
                for p in range(n_pair):
                    ska, skb = 2 * p, 2 * p + 1
                    diag = p >= n_pair - 2
                    r0 = 256 * (p - (n_pair - 2)) if diag else 0
                    # live sq-region offset per half (0 when not diagonal)
                    roffs = (r0, r0 + 128) if diag else (0, 0)
                    psS = [psA.tile([128, 1024], F32, tag="score", name="score")
                           for _ in range(2)]
                    for (sk, hbase, r) in ((ska, 0, roffs[0]),
                                           (skb, 512, roffs[1])):
                        ks = slice(128 * sk, 128 * (sk + 1))
                        qsr = slice(512 * j + r, 512 * (j + 1))
                        for hh in range(2):
                            rs = slice(64 * hh, 64 * (hh + 1))
                            nc.tensor.matmul(psS[hh][:, hbase + r:hbase + 512],
                                             KT[t][rs, ks], QT[t][rs, qsr],
                                             start=True, stop=True)
                    E = [epool.tile([128, 1024], F32R, tag="E", name="E")
                         for _ in range(2)]
                    for hh in range(2):
                        if not diag:
                            nc.scalar.activation(out=E[hh], in_=psS[hh],
                                                 func=EXP, scale=SCALE)
                        else:
                            for (hbase, r) in ((0, roffs[0]), (512, roffs[1])):
                                nc.scalar.activation(
                                    out=E[hh][:, hbase + r:hbase + 512],
                                    in_=psS[hh][:, hbase + r:hbase + 512],
                                    func=EXP, scale=SCALE)
                                if r > 0:
                                    nc.vector.tensor_copy(
                                        E[hh][:, hbase:hbase + r],
                                        zeros384[:, 0:r])
                                nc.vector.tensor_mul(
                                    E[hh][:, hbase + r:hbase + r + 128],
                                    E[hh][:, hbase + r:hbase + r + 128], tri)
                    for hh in range(2):
                        h = 2 * hp + hh
                        vc = slice(65 * (h % HPC), 65 * (h % HPC) + 65)
                        ra, rb = roffs
                        nc.tensor.matmul(pn[hh][:, ra:512], Vaug[ska][:, vc],
                                         E[hh][:, ra:512],
                                         start=(p == 0), stop=False)
                        nc.tensor.matmul(pn[hh][:, rb:512], Vaug[skb][:, vc],
                                         E[hh][:, 512 + rb:1024],
                                         start=False, stop=(p == n_pair - 1))
                # normalize -> attT
                for hh in range(2):
                    h = 2 * hp + hh
                    rs = slice(64 * hh, 64 * (hh + 1))
                    if opts["norm"] == "pe":
                        # den row -> SBUF via ACT, PE K=1 broadcast, recip, mult
                        dsb = ntmp.tile([1, 512], F32R, tag="dsb", name="dsb")
                        nc.scalar.copy(dsb, pn[hh][64:65, :])
                        bcp = psB.tile([64, 512], F32, tag="bcp", name="bcp", bufs=1)
                        nc.tensor.matmul(bcp, ones1, dsb, start=True, stop=True)
                        rc = ntmp.tile([64, 512], F32, tag="rc64", name="rc64")
                        nc.vector.reciprocal(rc, bcp)
                        nc.vector.tensor_mul(attT[t][rs, qs], pn[hh][0:64, :], rc)
                    else:
                        rc = ntmp.tile([1, 512], F32, tag="rc", name="rc")
                        nc.vector.reciprocal(rc, pn[hh][64:65, :])
                        drow = j * 4 + hp * 2 + hh
                        nc.sync.dma_start(out=den_dram[drow:drow + 1, :], in_=rc)
                        bc = ntmp.tile([64, 512], F32, tag="bc", name="bc")
                        nc.sync.dma_start(
                            out=bc,
                            in_=den_dram[drow:drow + 1, :].to_broadcast([64, 512]))
                        nc.vector.tensor_mul(attT[t][rs, qs], pn[hh][0:64, :], bc)

            # ---- partial Wo for this seq block ----
            for st in range(4):
                stg = 4 * j + st
                ss = slice(128 * stg, 128 * (stg + 1))
                row0 = 128 * st if use_rs else 128 * stg
                for ot in range(2):
                    os_ = slice(512 * ot, 512 * (ot + 1))
                    pw = psB.tile([128, 512], F32, tag="pw", name="pw", bufs=1)
                    for t in range(2):
                        nc.tensor.matmul(pw, attT[t][:, ss], wo_t[t][:, os_],
                                         start=(t == 0), stop=(t == 1))
                    ob = opool.tile([128, 512], F32, tag="ob", name="ob")
                    nc.vector.tensor_copy(ob, pw)
                    if use_rs:
                        nc.sync.dma_start(
                            out=cc_ins[j][row0:row0 + 128, os_], in_=ob)
                    else:
                        nc.sync.dma_start(out=out_d[ss, os_], in_=ob)
            if use_rs:
                nc.gpsimd.collective_compute(
                    "ReduceScatter", mybir.AluOpType.add,
                    ins=[cc_ins[j][:, :]], outs=[cc_outs[j][:, :]],
                    replica_groups=groups)
                nc.sync.dma_start(out=out_d[128 * j:128 * (j + 1), :],
                                  in_=cc_outs[j][:, :])


def get_program(use_rs=False, reps=1, **kw):
    opts = dict(DEFAULT_OPTS)
    opts.update(use_rs=use_rs, reps=reps, **kw)
    key = tuple(sorted(opts.items()))
    if key not in _CACHED:
        _CACHED[key] = _build_program(opts)
    return _CACHED[key]


def make_in_maps(x, Wq, Wk, Wv, Wo, token_positions):
    """Host-side sharding: per-core input dicts."""
    x = np.asarray(x, dtype=np.float32)
    Wq = np.asarray(Wq, dtype=np.float32)
    Wk = np.asarray(Wk, dtype=np.float32)
    Wv = np.asarray(Wv, dtype=np.float32)
    Wo = np.asarray(Wo, dtype=np.float32)
    pos = np.asarray(token_positions).astype(np.float32)

    # rope tables, feature-major [128, S]: row p -> pair index i = p % 32,
    # rows [0:32]=evens, [32:64]=odds per 64-row head block.
    i = np.arange(DH // 2, dtype=np.float32)
    d = THETA ** (2.0 * i / DH)                       # [32]
    tt = pos[None, :] / d[:, None]                    # [32, S]
    sin, cos = np.sin(tt), np.cos(tt)
    A = np.tile(cos, (4, 1)).astype(np.float32)       # [128, S]
    # B: evens row -> -sin, odds row -> +sin ; B2 = swap(B): evens->+sin, odds->-sin
    B2 = np.tile(np.concatenate([sin, -sin], axis=0), (2, 1)).astype(np.float32)

    # causal triangle mask [128, 128]: allow j >= p
    p = np.arange(128)[:, None]
    jj = np.arange(128)[None, :]
    tri = (jj >= p).astype(np.float32)

    # per-head Q/K row permutation: evens then odds
    i2 = np.arange(DH // 2)
    perm = np.concatenate(
        [np.concatenate([64 * h + 2 * i2, 64 * h + 2 * i2 + 1]) for h in range(H)])

    in_maps = []
    for c in range(CORES):
        b, g = c // 4, c % 4
        rows = perm[F * g:F * (g + 1)]
        nat = np.arange(F * g, F * (g + 1))
        in_maps.append({
            "xT": np.ascontiguousarray(x[b].T),
            "wqT": np.ascontiguousarray(Wq[rows, :].T),
            "wkT": np.ascontiguousarray(Wk[rows, :].T),
            "wvT": np.ascontiguousarray(Wv[nat, :].T),
            "woT": np.ascontiguousarray(Wo[:, nat].T),
            "ropeA": A,
            "ropeB2": B2,
            "tri": tri,
        })
    return in_maps


def assemble(results, use_rs=False):
    out = np.empty((B, S, D), dtype=np.float32)
    if use_rs:
        for b in range(B):
            for g in range(4):
                r = results[4 * b + g]["out_rs"]
                # out_rs rows: for each seq block j, rows 128j..128j+128
                # correspond to global rows 512*j + 128*g .. +128
                for j in range(NSB):
                    out[b, 512 * j + 128 * g:512 * j + 128 * (g + 1), :] = \
                        r[128 * j:128 * (j + 1), :]
    else:
        for b in range(B):
            acc = results[4 * b]["partial"].astype(np.float32).copy()
            for g in range(1, 4):
                acc += results[4 * b + g]["partial"]
            out[b] = acc
    return out


def kernel(x, Wq, Wk, Wv, Wo, token_positions):
    from concourse.bass_utils import run_bass_kernel_spmd
    use_rs = bool(int(os.environ.get("MHA_USE_RS", "0")))
    nc = get_program(use_rs)
    in_maps = make_in_maps(x, Wq, Wk, Wv, Wo, token_positions)
    res = run_bass_kernel_spmd(nc, in_maps, list(range(CORES)))
    return assemble(res.results, use_rs)



# revision 28
# speedup vs baseline: 22.1607x; 22.1607x over previous
"""Multi-head self-attention (RoPE, causal) Trainium2 Bass kernel.

Sharding: 8 cores = 2 batches x 4 head-groups (4 heads each).
Each core computes QKV projections for its heads (feature-major via x^T),
RoPE, causal attention with transposed scores (softmax along partitions
handled via exp + ones-column denominator in the V matmul), and a partial
output projection over its head slice. The host sums the 4 partials per
batch (reduce step of the tensor-parallel output projection).

v2 over the baseline:
- rope even/odd swap via PE permutation matmul (no per-tile swap DMAs)
- packed single-DMA weight loads, column-grouped x loads (early PE start)
- bf16 QT/KT/E/Vaug/attT/wo (f32r kept on the projection path)
- attention block j software-pipelined with proj(j+1) and Wo(j-1)
- batched softmax normalization (one broadcast matmul per (j,hp))
- rope mul/add on the idle GpSimd engine
"""
import math

from contextlib import ExitStack

import numpy as np

import concourse.tile as tile
from concourse import bacc, mybir

F32 = mybir.dt.float32
F32R = mybir.dt.float32r
BF16 = mybir.dt.bfloat16
EXP = mybir.ActivationFunctionType.Exp

B, S, D, H, DH = 2, 2048, 1024, 16, 64
THETA = 10000.0
CORES = 8
HPC = 4                    # heads per core
F = HPC * DH               # 256 features per core
SCALE = 1.0 / math.sqrt(DH)
NKT = D // 128             # 8 k tiles
NSB = S // 512             # 4 seq blocks of 512
NST = S // 128             # 16 seq tiles of 128

DEFAULT_OPTS = dict(reps=1, ilv=True, rope_pool=True)

_CACHED = {}


def _build_program(opts):
    reps = opts["reps"]
    nc = bacc.Bacc("TRN2", target_bir_lowering=False, debug=False,
                   num_devices=CORES)

    xT = nc.dram_tensor("xT", [D, S], BF16, kind="ExternalInput")
    wqP_d = nc.dram_tensor("wqP", [128, 2048], BF16, kind="ExternalInput")
    wkP_d = nc.dram_tensor("wkP", [128, 2048], BF16, kind="ExternalInput")
    wvP_d = nc.dram_tensor("wvP", [128, 2048], BF16, kind="ExternalInput")
    woP_d = nc.dram_tensor("woP", [128, 2048], BF16, kind="ExternalInput")
    ropeA_d = nc.dram_tensor("ropeA", [128, S], BF16, kind="ExternalInput")
    ropeB_d = nc.dram_tensor("ropeB", [128, S], BF16, kind="ExternalInput")
    tri_d = nc.dram_tensor("tri", [128, 128], BF16, kind="ExternalInput")
    P_d = nc.dram_tensor("Pm", [128, 128], F32R, kind="ExternalInput")
    Msel_d = nc.dram_tensor("Msel", [33, 128], BF16, kind="ExternalInput")

    out_d = nc.dram_tensor("partial", [S, D], BF16, kind="ExternalOutput")

    with tile.TileContext(nc) as tc, ExitStack() as ctx:
        persist = ctx.enter_context(tc.tile_pool(name="persist", bufs=1))

        xt = [persist.tile([128, S], BF16, tag=f"xt{k}", name=f"xt{k}")
              for k in range(NKT)]
        wq = persist.tile([128, 2048], BF16, tag="wq", name="wq")
        wk = persist.tile([128, 2048], BF16, tag="wk", name="wk")
        wv = persist.tile([128, 2048], BF16, tag="wv", name="wv")
        wo = persist.tile([128, 2048], BF16, tag="wo", name="wo")
        ropeA = persist.tile([128, S], BF16, tag="ropeA", name="ropeA")
        ropeB = persist.tile([128, S], BF16, tag="ropeB", name="ropeB")
        tri = persist.tile([128, 128], BF16, tag="tri", name="tri")
        Pm = persist.tile([128, 128], F32R, tag="Pm", name="Pm")
        Msel = persist.tile([33, 128], BF16, tag="Msel", name="Msel")
        QT = [persist.tile([128, S], BF16, tag=f"QT{t}", name=f"QT{t}") for t in range(2)]
        KT = [persist.tile([128, S], BF16, tag=f"KT{t}", name=f"KT{t}") for t in range(2)]
        Vaug = [persist.tile([128, 260], BF16, tag=f"Vaug{st}", name=f"Vaug{st}")
                for st in range(NST)]
        attT = [persist.tile([128, S], BF16, tag=f"attT{t}", name=f"attT{t}") for t in range(2)]
        dsb2 = persist.tile([33, 512], BF16, tag="dsb2", name="dsb2")
        zf32 = persist.tile([128, 512], F32, tag="zf32", name="zf32")
        nc.vector.memset(zf32, 0.0)
        nc.vector.tensor_copy(dsb2, zf32[0:33, :])
        zeros384 = persist.tile([128, 384], BF16, tag="zeros384", name="zeros384")
        nc.vector.tensor_copy(zeros384, zf32[:, 0:384])

        # ones columns of Vaug: written once, never touched by the rep body
        ones4f = persist.tile([128, 4], F32, tag="ones4f", name="ones4f")
        nc.vector.memset(ones4f, 1.0)
        for st in range(NST):
            nc.vector.tensor_copy(Vaug[st][:, 64:260:65], ones4f)

        # ---- loads ----
        # secondary tensors via SWDGE on the idle Pool engine (parallel
        # dispatch queue); the x/wq/wk stream stays on HWDGE.
        ld = nc.sync.dma_start
        ld(out=wq[:, 0:1024], in_=wqP_d[:, 0:1024])
        for k in range(NKT):
            ld(out=xt[k][:, 0:512], in_=xT[128 * k:128 * (k + 1), 0:512])
        sld = nc.gpsimd.dma_start
        sld(out=ropeA, in_=ropeA_d[:, :])
        sld(out=ropeB, in_=ropeB_d[:, :])
        sld(out=Pm, in_=P_d[:, :])
        sld(out=wv, in_=wvP_d[:, :])
        sld(out=tri, in_=tri_d[:, :])
        sld(out=Msel, in_=Msel_d[:, :])
        ld(out=wq[:, 1024:2048], in_=wqP_d[:, 1024:2048])
        ld(out=wk[:, 0:1024], in_=wkP_d[:, 0:1024])
        ld(out=wk[:, 1024:2048], in_=wkP_d[:, 1024:2048])
        for k in range(NKT):
            ld(out=xt[k][:, 512:1024], in_=xT[128 * k:128 * (k + 1), 512:1024])
        for k in range(NKT):
            ld(out=xt[k][:, 1024:2048], in_=xT[128 * k:128 * (k + 1), 1024:2048])
        ld(out=wo, in_=woP_d[:, :])

        env = dict(xt=xt, wq=wq, wk=wk, wv=wv, wo=wo, ropeA=ropeA,
                   ropeB=ropeB, tri=tri, Pm=Pm, Msel=Msel, QT=QT, KT=KT,
                   Vaug=Vaug, attT=attT, out_d=out_d, dsb2=dsb2,
                   zeros384=zeros384)
        for _rep in range(reps):
            _body(nc, tc, opts, env)

    nc.compile()
    return nc


def _body(nc, tc, opts, env):
    xt = env["xt"]; wq = env["wq"]; wk = env["wk"]; wv = env["wv"]
    wo = env["wo"]; ropeA = env["ropeA"]; ropeB = env["ropeB"]
    tri = env["tri"]; Pm = env["Pm"]; Msel = env["Msel"]
    QT = env["QT"]; KT = env["KT"]; Vaug = env["Vaug"]; attT = env["attT"]
    out_d = env["out_d"]; dsb2 = env["dsb2"]; zeros384 = env["zeros384"]
    rope_mul = nc.gpsimd.tensor_mul if opts["rope_pool"] else nc.vector.tensor_mul
    rope_add = nc.gpsimd.tensor_add if opts["rope_pool"] else nc.vector.tensor_add

    with tc.tile_pool(name="rp", bufs=6) as rp, \
         tc.tile_pool(name="oap", bufs=4) as oap, \
         tc.tile_pool(name="ep", bufs=12) as ep, \
         tc.tile_pool(name="np", bufs=3) as np_, \
         tc.tile_pool(name="op", bufs=3) as op, \
         tc.tile_pool(name="psum", bufs=2, space="PSUM") as psum:

        def qkproj_a(sb, w, t):
            cs = slice(512 * sb, 512 * (sb + 1))
            ps = psum.tile([128, 512], F32, tag="sh", name="proj")
            for k in range(NKT):
                nc.tensor.matmul(ps, w[:, 1024 * t + 128 * k:1024 * t + 128 * (k + 1)],
                                 xt[k][:, cs], start=(k == 0), stop=(k == NKT - 1))
            raw = rp.tile([128, 512], F32R, tag="raw", name="raw")
            nc.vector.tensor_copy(raw, ps)
            return raw

        def qkproj_b(sb, dest, t, raw):
            cs = slice(512 * sb, 512 * (sb + 1))
            rmul = nc.vector.tensor_mul if sb == 0 else rope_mul
            radd = nc.vector.tensor_add if sb == 0 else rope_add
            sw = psum.tile([128, 512], F32, tag="sh", name="projsw")
            nc.tensor.matmul(sw, Pm, raw, start=True, stop=True)
            t1 = rp.tile([128, 512], F32, tag="t1", name="t1")
            rmul(t1, raw, ropeA[:, cs])
            t2 = rp.tile([128, 512], F32, tag="t2", name="t2")
            nc.vector.tensor_mul(t2, sw, ropeB[:, cs])
            radd(dest[t][:, cs], t1, t2)

        def vproj(st):
            ss = slice(128 * st, 128 * (st + 1))
            ps = psum.tile([128, 512], F32, tag="sh", name="projv")
            for k in range(NKT):
                nc.tensor.matmul(ps[:, 0:256], xt[k][:, ss],
                                 wv[:, 256 * k:256 * (k + 1)],
                                 start=(k == 0), stop=(k == NKT - 1))
            dst = Vaug[st][:, 0:260].rearrange("p (h c) -> p h c", h=HPC)
            nc.vector.tensor_copy(dst[:, :, 0:64],
                                  ps[:, 0:256].rearrange("p (h c) -> p h c", h=HPC))

        def wo_chunk(j, st):
            stg = 4 * j + st
            ss = slice(128 * stg, 128 * (stg + 1))
            ob = op.tile([128, 1024], BF16, tag="ob", name="ob")
            for ot in range(2):
                os_ = slice(512 * ot, 512 * (ot + 1))
                pw = psum.tile([128, 512], F32, tag="sh", name="pw")
                for t in range(2):
                    nc.tensor.matmul(pw, attT[t][:, ss],
                                     wo[:, 1024 * t + 512 * ot:1024 * t + 512 * (ot + 1)],
                                     start=(t == 0), stop=(t == 1))
                nc.vector.tensor_copy(ob[:, os_], pw)
            nc.sync.dma_start(out=out_d[ss, :], in_=ob)

        def att_block(j, fill0, fill1):
            qs = slice(512 * j, 512 * (j + 1))
            n_pair = 2 * (j + 1)

            for hp in range(2):
                t = hp
                fillers = fill0 if hp == 0 else fill1
                emitted = 0
                stage = 0

                def fill():
                    nonlocal emitted, stage
                    stage += 1
                    want = len(fillers) * stage // n_pair
                    while emitted < want:
                        fillers[emitted]()
                        emitted += 1
                pn = [psum.tile([65, 512], F32, tag="pn", name="pn")
                      for _ in range(2)]
                pending = None

                def attv(p, E, roffs):
                    for hh in range(2):
                        h = 2 * hp + hh
                        vc = slice(65 * (h % HPC), 65 * (h % HPC) + 65)
                        ra, rb = roffs
                        nc.tensor.matmul(pn[hh][:, ra:512],
                                         Vaug[2 * p][:, vc],
                                         E[hh][:, ra:512],
                                         start=(p == 0), stop=False)
                        nc.tensor.matmul(pn[hh][:, rb:512],
                                         Vaug[2 * p + 1][:, vc],
                                         E[hh][:, 512 + rb:1024],
                                         start=False, stop=(p == n_pair - 1))

                for p in range(n_pair):
                    ska, skb = 2 * p, 2 * p + 1
                    diag = p >= n_pair - 2
                    r0 = 256 * (p - (n_pair - 2)) if diag else 0
                    roffs = (r0, r0 + 128) if diag else (0, 0)
                    psS = [psum.tile([128, 1024], F32, tag="score", name="score")
                           for _ in range(2)]
                    for (sk, hbase, r) in ((ska, 0, roffs[0]),
                                           (skb, 512, roffs[1])):
                        ks = slice(128 * sk, 128 * (sk + 1))
                        qsr = slice(512 * j + r, 512 * (j + 1))
                        for hh in range(2):
                            rs = slice(64 * hh, 64 * (hh + 1))
                            nc.tensor.matmul(psS[hh][:, hbase + r:hbase + 512],
                                             KT[t][rs, ks], QT[t][rs, qsr],
                                             start=True, stop=True)
                    if pending is not None:
                        attv(*pending)
                    E = [ep.tile([128, 1024], BF16, tag="E", name="E")
                         for _ in range(2)]
                    for hh in range(2):
                        if not diag:
                            nc.scalar.activation(out=E[hh], in_=psS[hh],
                                                 func=EXP, scale=SCALE)
                        else:
                            for (hbase, r) in ((0, roffs[0]), (512, roffs[1])):
                                nc.scalar.activation(
                                    out=E[hh][:, hbase + r:hbase + 512],
                                    in_=psS[hh][:, hbase + r:hbase + 512],
                                    func=EXP, scale=SCALE)
                                if r > 0:
                                    nc.gpsimd.tensor_copy(
                                        E[hh][:, hbase:hbase + r],
                                        zeros384[:, 0:r])
                                nc.vector.tensor_mul(
                                    E[hh][:, hbase + r:hbase + r + 128],
                                    E[hh][:, hbase + r:hbase + r + 128], tri)
                    pending = (p, E, roffs)
                    fill()
                attv(*pending)
                # batched normalize for this head pair
                nc.scalar.copy(dsb2[0:1, :], pn[0][64:65, :])
                nc.scalar.copy(dsb2[32:33, :], pn[1][64:65, :])
                bcp = psum.tile([128, 512], F32, tag="sh", name="bcp")
                nc.tensor.matmul(bcp, Msel, dsb2, start=True, stop=True)
                rc = np_.tile([128, 512], F32, tag="rc", name="rc")
                nc.vector.reciprocal(rc, bcp)
                for hh in range(2):
                    rs = slice(64 * hh, 64 * (hh + 1))
                    nc.vector.tensor_mul(attT[t][rs, qs], pn[hh][0:64, :],
                                         rc[rs, :])
                while emitted < len(fillers):
                    fillers[emitted]()
                    emitted += 1

        def wo_half_a(j, st):
            stg = 4 * j + st
            ss = slice(128 * stg, 128 * (stg + 1))
            oba = oap.tile([128, 1024], F32, tag="oba", name="oba")
            for ot in range(2):
                os_ = slice(512 * ot, 512 * (ot + 1))
                pw = psum.tile([128, 512], F32, tag="sh", name="pwa")
                nc.tensor.matmul(pw, attT[0][:, ss], wo[:, 512 * ot:512 * (ot + 1)],
                                 start=True, stop=True)
                nc.vector.tensor_copy(oba[:, os_], pw)
            return oba

        def wo_half_b(j, st, oba):
            stg = 4 * j + st
            ss = slice(128 * stg, 128 * (stg + 1))
            ob = op.tile([128, 1024], BF16, tag="ob", name="ob")
            pw = psum.tile([128, 1024], F32, tag="score", name="pwb")
            for ot in range(2):
                os_ = slice(512 * ot, 512 * (ot + 1))
                nc.tensor.matmul(pw[:, os_], attT[1][:, ss],
                                 wo[:, 1024 + 512 * ot:1024 + 512 * (ot + 1)],
                                 start=True, stop=True)
            nc.vector.tensor_add(ob, pw, oba)
            nc.sync.dma_start(out=out_d[ss, :], in_=ob)

        ilv = opts["ilv"]
        obas = {}

        def proj_fillers(sb):
            fl = []
            for (w, dest) in ((wq, QT), (wk, KT)):
                for t in range(2):
                    box = {}
                    fl.append(lambda sb=sb, w=w, t=t, box=box:
                              box.__setitem__("raw", qkproj_a(sb, w, t)))
                    fl.append(lambda sb=sb, dest=dest, t=t, box=box:
                              qkproj_b(sb, dest, t, box["raw"]))
            for st4 in range(4):
                fl.append(lambda st=4 * sb + st4: vproj(st))
            return fl

        for i in range(NSB + 2):
            fill0, fill1 = [], []
            if i < NSB:
                fl = proj_fillers(i)
                fill0, fill1 = fl[:8], fl[8:]
            if i == NSB:
                # last block: PE is starved while ACT grinds exp, so all the
                # deferred Wo work plus the attT[0]-half of Wo(j3) goes here
                fill0 = [lambda j=j, st=st: wo_chunk(j, st)
                         for j in range(2) for st in range(4)]
                fill1 = ([lambda st=st: wo_chunk(2, st) for st in range(4)] +
                         [lambda st=st: obas.__setitem__(st, wo_half_a(NSB - 1, st))
                          for st in range(4)])
            if i == NSB + 1:
                for st in range(4):
                    wo_half_b(NSB - 1, st, obas[st])
                continue
            if 1 <= i <= NSB and ilv:
                att_block(i - 1, fill0, fill1)
            else:
                for f in fill0 + fill1:
                    f()
                if 1 <= i <= NSB:
                    att_block(i - 1, [], [])


def get_program(use_rs=False, reps=1, **kw):
    opts = dict(DEFAULT_OPTS)
    opts.update(reps=reps, **kw)
    key = tuple(sorted(opts.items()))
    if key not in _CACHED:
        _CACHED[key] = _build_program(opts)
    return _CACHED[key]


def make_in_maps(x, Wq, Wk, Wv, Wo, token_positions):
    """Host-side sharding: per-core input dicts."""
    import ml_dtypes
    bf16 = ml_dtypes.bfloat16
    x = np.asarray(x, dtype=np.float32)
    Wq = np.asarray(Wq, dtype=np.float32)
    Wk = np.asarray(Wk, dtype=np.float32)
    Wv = np.asarray(Wv, dtype=np.float32)
    Wo = np.asarray(Wo, dtype=np.float32)
    pos = np.asarray(token_positions).astype(np.float32)

    # rope tables, feature-major [128, S]: row p -> pair index i = p % 32,
    # rows [0:32]=evens, [32:64]=odds per 64-row head block.
    i = np.arange(DH // 2, dtype=np.float32)
    d = THETA ** (2.0 * i / DH)                       # [32]
    tt = pos[None, :] / d[:, None]                    # [32, S]
    sin, cos = np.sin(tt), np.cos(tt)
    A = np.tile(cos, (4, 1)).astype(bf16)             # [128, S]
    # dest = raw*A + swap(raw)*B ; B: evens row -> -sin, odds row -> +sin
    Bt = np.tile(np.concatenate([-sin, sin], axis=0), (2, 1)).astype(bf16)

    # causal triangle mask [128, 128]: allow j >= p
    p = np.arange(128)[:, None]
    jj = np.arange(128)[None, :]
    tri = (jj >= p).astype(bf16)

    # PE swap permutation: out row i = raw row (i ^ 32)
    Pm = np.zeros((128, 128), np.float32)
    Pm[np.arange(128) ^ 32, np.arange(128)] = 1.0

    # normalize broadcast selector: den rows at partitions 0 and 32
    Msel = np.zeros((33, 128), bf16)
    Msel[0, 0:64] = 1.0
    Msel[32, 64:128] = 1.0

    # per-head Q/K row permutation: evens then odds
    i2 = np.arange(DH // 2)
    perm = np.concatenate(
        [np.concatenate([64 * h + 2 * i2, 64 * h + 2 * i2 + 1]) for h in range(H)])

    def pack8(wT):  # [1024, 256] -> [128, 2048] with k chunks side by side
        return np.concatenate([wT[128 * k:128 * (k + 1), :] for k in range(NKT)],
                              axis=1)

    def pack8t(wT):  # [1024, 256] -> [128, 2048], t-major: [t][k][128]
        return np.concatenate(
            [wT[128 * k:128 * (k + 1), 128 * t:128 * (t + 1)]
             for t in range(2) for k in range(NKT)], axis=1)

    def pack2(wT):  # [256, 1024] -> [128, 2048]
        return np.concatenate([wT[128 * t:128 * (t + 1), :] for t in range(2)],
                              axis=1)

    in_maps = []
    for c in range(CORES):
        b, g = c // 4, c % 4
        rows = perm[F * g:F * (g + 1)]
        nat = np.arange(F * g, F * (g + 1))
        in_maps.append({
            "xT": np.ascontiguousarray(x[b].T).astype(bf16),
            "wqP": np.ascontiguousarray(pack8t(Wq[rows, :].T)).astype(bf16),
            "wkP": np.ascontiguousarray(pack8t(Wk[rows, :].T)).astype(bf16),
            "wvP": np.ascontiguousarray(pack8(Wv[nat, :].T)).astype(bf16),
            "woP": np.ascontiguousarray(pack2(Wo[:, nat].T).astype(bf16)),
            "ropeA": A,
            "ropeB": Bt,
            "tri": tri,
            "Pm": Pm,
            "Msel": Msel,
        })
    return in_maps


def assemble(results, use_rs=False):
    out = np.empty((B, S, D), dtype=np.float32)
    for b in range(B):
        acc = results[4 * b]["partial"].astype(np.float32).copy()
        for g in range(1, 4):
            acc += results[4 * b + g]["partial"]
        out[b] = acc
    return out


def kernel(x, Wq, Wk, Wv, Wo, token_positions):
    from concourse.bass_utils import run_bass_kernel_spmd
    nc = get_program(False)
    in_maps = make_in_maps(x, Wq, Wk, Wv, Wo, token_positions)
    res = run_bass_kernel_spmd(nc, in_maps, list(range(CORES)))
    return assemble(res.results)


# revision 31
# speedup vs baseline: 22.2923x; 1.0059x over previous
"""Multi-head self-attention (RoPE, causal) Trainium2 Bass kernel.

Sharding: 8 cores = 2 batches x 4 head-groups (4 heads each).
Each core computes QKV projections for its heads (feature-major via x^T),
RoPE, causal attention with transposed scores (softmax along partitions
handled via exp + ones-column denominator in the V matmul), and a partial
output projection over its head slice. The host sums the 4 partials per
batch (reduce step of the tensor-parallel output projection).

v2 over the baseline:
- rope even/odd swap via PE permutation matmul (no per-tile swap DMAs)
- packed single-DMA weight loads, column-grouped x loads (early PE start)
- bf16 QT/KT/E/Vaug/attT/wo (f32r kept on the projection path)
- attention block j software-pipelined with proj(j+1) and Wo(j-1)
- batched softmax normalization (one broadcast matmul per (j,hp))
- rope mul/add on the idle GpSimd engine
"""
import math

from contextlib import ExitStack

import numpy as np

import concourse.tile as tile
from concourse import bacc, mybir

F32 = mybir.dt.float32
F32R = mybir.dt.float32r
BF16 = mybir.dt.bfloat16
EXP = mybir.ActivationFunctionType.Exp

B, S, D, H, DH = 2, 2048, 1024, 16, 64
THETA = 10000.0
CORES = 8
HPC = 4                    # heads per core
F = HPC * DH               # 256 features per core
SCALE = 1.0 / math.sqrt(DH)
NKT = D // 128             # 8 k tiles
NSB = S // 512             # 4 seq blocks of 512
NST = S // 128             # 16 seq tiles of 128

DEFAULT_OPTS = dict(reps=1, ilv=True, rope_pool=True)

_CACHED = {}


def _build_program(opts):
    reps = opts["reps"]
    nc = bacc.Bacc("TRN2", target_bir_lowering=False, debug=False,
                   num_devices=CORES)

    xT = nc.dram_tensor("xT", [D, S], BF16, kind="ExternalInput")
    wqP_d = nc.dram_tensor("wqP", [128, 2048], BF16, kind="ExternalInput")
    wkP_d = nc.dram_tensor("wkP", [128, 2048], BF16, kind="ExternalInput")
    wvP_d = nc.dram_tensor("wvP", [128, 2048], BF16, kind="ExternalInput")
    woP_d = nc.dram_tensor("woP", [128, 2048], BF16, kind="ExternalInput")
    ropeA_d = nc.dram_tensor("ropeA", [128, S], BF16, kind="ExternalInput")
    ropeB_d = nc.dram_tensor("ropeB", [128, S], BF16, kind="ExternalInput")
    tri_d = nc.dram_tensor("tri", [128, 128], BF16, kind="ExternalInput")
    P_d = nc.dram_tensor("Pm", [128, 128], F32R, kind="ExternalInput")
    Msel_d = nc.dram_tensor("Msel", [33, 128], BF16, kind="ExternalInput")

    out_d = nc.dram_tensor("partial", [S, D], BF16, kind="ExternalOutput")

    with tile.TileContext(nc) as tc, ExitStack() as ctx:
        persist = ctx.enter_context(tc.tile_pool(name="persist", bufs=1))

        xt = [persist.tile([128, S], BF16, tag=f"xt{k}", name=f"xt{k}")
              for k in range(NKT)]
        wq = persist.tile([128, 2048], BF16, tag="wq", name="wq")
        wk = persist.tile([128, 2048], BF16, tag="wk", name="wk")
        wv = persist.tile([128, 2048], BF16, tag="wv", name="wv")
        wo = persist.tile([128, 2048], BF16, tag="wo", name="wo")
        ropeA = persist.tile([128, S], BF16, tag="ropeA", name="ropeA")
        ropeB = persist.tile([128, S], BF16, tag="ropeB", name="ropeB")
        tri = persist.tile([128, 128], BF16, tag="tri", name="tri")
        Pm = persist.tile([128, 128], F32R, tag="Pm", name="Pm")
        Msel = persist.tile([33, 128], BF16, tag="Msel", name="Msel")
        QT = [persist.tile([128, S], BF16, tag=f"QT{t}", name=f"QT{t}") for t in range(2)]
        KT = [persist.tile([128, S], BF16, tag=f"KT{t}", name=f"KT{t}") for t in range(2)]
        Vaug = [persist.tile([128, 260], BF16, tag=f"Vaug{st}", name=f"Vaug{st}")
                for st in range(NST)]
        attT = [persist.tile([128, S], BF16, tag=f"attT{t}", name=f"attT{t}") for t in range(2)]
        dsb2 = persist.tile([33, 512], BF16, tag="dsb2", name="dsb2")
        zf32 = persist.tile([128, 512], F32, tag="zf32", name="zf32")
        nc.vector.memset(zf32, 0.0)
        nc.vector.tensor_copy(dsb2, zf32[0:33, :])
        zeros384 = persist.tile([128, 384], BF16, tag="zeros384", name="zeros384")
        nc.vector.tensor_copy(zeros384, zf32[:, 0:384])

        # ones columns of Vaug: written once, never touched by the rep body
        ones4f = persist.tile([128, 4], F32, tag="ones4f", name="ones4f")
        nc.vector.memset(ones4f, 1.0)
        for st in range(NST):
            nc.vector.tensor_copy(Vaug[st][:, 64:260:65], ones4f)

        # ---- loads ----
        # secondary tensors via SWDGE on the idle Pool engine (parallel
        # dispatch queue); the x/wq/wk stream stays on HWDGE.
        ld = nc.sync.dma_start
        ld(out=wq[:, 0:1024], in_=wqP_d[:, 0:1024])
        for k in range(NKT):
            ld(out=xt[k][:, 0:512], in_=xT[128 * k:128 * (k + 1), 0:512])
        sld = nc.gpsimd.dma_start
        sld(out=ropeA, in_=ropeA_d[:, :])
        sld(out=ropeB, in_=ropeB_d[:, :])
        sld(out=Pm, in_=P_d[:, :])
        sld(out=wv, in_=wvP_d[:, :])
        sld(out=tri, in_=tri_d[:, :])
        sld(out=Msel, in_=Msel_d[:, :])
        ld(out=wq[:, 1024:2048], in_=wqP_d[:, 1024:2048])
        ld(out=wk[:, 0:1024], in_=wkP_d[:, 0:1024])
        ld(out=wk[:, 1024:2048], in_=wkP_d[:, 1024:2048])
        for k in range(NKT):
            ld(out=xt[k][:, 512:1024], in_=xT[128 * k:128 * (k + 1), 512:1024])
        for k in range(NKT):
            ld(out=xt[k][:, 1024:2048], in_=xT[128 * k:128 * (k + 1), 1024:2048])
        ld(out=wo, in_=woP_d[:, :])

        env = dict(xt=xt, wq=wq, wk=wk, wv=wv, wo=wo, ropeA=ropeA,
                   ropeB=ropeB, tri=tri, Pm=Pm, Msel=Msel, QT=QT, KT=KT,
                   Vaug=Vaug, attT=attT, out_d=out_d, dsb2=dsb2,
                   zeros384=zeros384)
        for _rep in range(reps):
            _body(nc, tc, opts, env)

    nc.compile()
    return nc


def _body(nc, tc, opts, env):
    xt = env["xt"]; wq = env["wq"]; wk = env["wk"]; wv = env["wv"]
    wo = env["wo"]; ropeA = env["ropeA"]; ropeB = env["ropeB"]
    tri = env["tri"]; Pm = env["Pm"]; Msel = env["Msel"]
    QT = env["QT"]; KT = env["KT"]; Vaug = env["Vaug"]; attT = env["attT"]
    out_d = env["out_d"]; dsb2 = env["dsb2"]; zeros384 = env["zeros384"]
    rope_mul = nc.gpsimd.tensor_mul if opts["rope_pool"] else nc.vector.tensor_mul
    rope_add = nc.gpsimd.tensor_add if opts["rope_pool"] else nc.vector.tensor_add

    with tc.tile_pool(name="rp", bufs=6) as rp, \
         tc.tile_pool(name="oap", bufs=4) as oap, \
         tc.tile_pool(name="ep", bufs=12) as ep, \
         tc.tile_pool(name="np", bufs=3) as np_, \
         tc.tile_pool(name="op", bufs=3) as op, \
         tc.tile_pool(name="psum", bufs=2, space="PSUM") as psum:

        def qkproj_a(sb, w, t):
            cs = slice(512 * sb, 512 * (sb + 1))
            ps = psum.tile([128, 512], F32, tag="sh", name="proj")
            for k in range(NKT):
                nc.tensor.matmul(ps, w[:, 1024 * t + 128 * k:1024 * t + 128 * (k + 1)],
                                 xt[k][:, cs], start=(k == 0), stop=(k == NKT - 1))
            raw = rp.tile([128, 512], F32R, tag="raw", name="raw")
            nc.vector.tensor_copy(raw, ps)
            return raw

        def qkproj_b(sb, dest, t, raw):
            cs = slice(512 * sb, 512 * (sb + 1))
            rmul = nc.vector.tensor_mul if sb == 0 else rope_mul
            radd = nc.vector.tensor_add if sb == 0 else rope_add
            sw = psum.tile([128, 512], F32, tag="sh", name="projsw")
            nc.tensor.matmul(sw, Pm, raw, start=True, stop=True)
            t1 = rp.tile([128, 512], F32, tag="t1", name="t1")
            rmul(t1, raw, ropeA[:, cs])
            t2 = rp.tile([128, 512], F32, tag="t2", name="t2")
            nc.vector.tensor_mul(t2, sw, ropeB[:, cs])
            radd(dest[t][:, cs], t1, t2)

        def vproj(st):
            ss = slice(128 * st, 128 * (st + 1))
            ps = psum.tile([128, 512], F32, tag="sh", name="projv")
            for k in range(NKT):
                nc.tensor.matmul(ps[:, 0:256], xt[k][:, ss],
                                 wv[:, 256 * k:256 * (k + 1)],
                                 start=(k == 0), stop=(k == NKT - 1))
            dst = Vaug[st][:, 0:260].rearrange("p (h c) -> p h c", h=HPC)
            nc.vector.tensor_copy(dst[:, :, 0:64],
                                  ps[:, 0:256].rearrange("p (h c) -> p h c", h=HPC))

        def wo_chunk(j, st):
            stg = 4 * j + st
            ss = slice(128 * stg, 128 * (stg + 1))
            ob = op.tile([128, 1024], BF16, tag="ob", name="ob")
            for ot in range(2):
                os_ = slice(512 * ot, 512 * (ot + 1))
                pw = psum.tile([128, 512], F32, tag="sh", name="pw")
                for t in range(2):
                    nc.tensor.matmul(pw, attT[t][:, ss],
                                     wo[:, 1024 * t + 512 * ot:1024 * t + 512 * (ot + 1)],
                                     start=(t == 0), stop=(t == 1))
                nc.vector.tensor_copy(ob[:, os_], pw)
            nc.sync.dma_start(out=out_d[ss, :], in_=ob)

        def att_block(j, fill0, fill1):
            qs = slice(512 * j, 512 * (j + 1))
            n_pair = 2 * (j + 1)

            for hp in range(2):
                t = hp
                fillers = fill0 if hp == 0 else fill1
                emitted = 0
                stage = 0

                def fill():
                    nonlocal emitted, stage
                    stage += 1
                    lo = n_pair // 3
                    want = (0 if stage <= lo else
                            len(fillers) * (stage - lo) // (n_pair - lo))
                    while emitted < want:
                        fillers[emitted]()
                        emitted += 1
                pn = [psum.tile([65, 512], F32, tag="pn", name="pn")
                      for _ in range(2)]
                pending = None

                def attv(p, E, roffs):
                    for hh in range(2):
                        h = 2 * hp + hh
                        vc = slice(65 * (h % HPC), 65 * (h % HPC) + 65)
                        ra, rb = roffs
                        nc.tensor.matmul(pn[hh][:, ra:512],
                                         Vaug[2 * p][:, vc],
                                         E[hh][:, ra:512],
                                         start=(p == 0), stop=False)
                        nc.tensor.matmul(pn[hh][:, rb:512],
                                         Vaug[2 * p + 1][:, vc],
                                         E[hh][:, 512 + rb:1024],
                                         start=False, stop=(p == n_pair - 1))

                for p in range(n_pair):
                    ska, skb = 2 * p, 2 * p + 1
                    diag = p >= n_pair - 2
                    r0 = 256 * (p - (n_pair - 2)) if diag else 0
                    roffs = (r0, r0 + 128) if diag else (0, 0)
                    psS = [psum.tile([128, 1024], F32, tag="score", name="score")
                           for _ in range(2)]
                    for (sk, hbase, r) in ((ska, 0, roffs[0]),
                                           (skb, 512, roffs[1])):
                        ks = slice(128 * sk, 128 * (sk + 1))
                        qsr = slice(512 * j + r, 512 * (j + 1))
                        for hh in range(2):
                            rs = slice(64 * hh, 64 * (hh + 1))
                            nc.tensor.matmul(psS[hh][:, hbase + r:hbase + 512],
                                             KT[t][rs, ks], QT[t][rs, qsr],
                                             start=True, stop=True)
                    if pending is not None:
                        attv(*pending)
                    E = [ep.tile([128, 1024], BF16, tag="E", name="E")
                         for _ in range(2)]
                    for hh in range(2):
                        if not diag:
                            nc.scalar.activation(out=E[hh], in_=psS[hh],
                                                 func=EXP, scale=SCALE)
                        else:
                            for (hbase, r) in ((0, roffs[0]), (512, roffs[1])):
                                nc.scalar.activation(
                                    out=E[hh][:, hbase + r:hbase + 512],
                                    in_=psS[hh][:, hbase + r:hbase + 512],
                                    func=EXP, scale=SCALE)
                                if r > 0:
                                    nc.gpsimd.tensor_copy(
                                        E[hh][:, hbase:hbase + r],
                                        zeros384[:, 0:r])
                                nc.vector.tensor_mul(
                                    E[hh][:, hbase + r:hbase + r + 128],
                                    E[hh][:, hbase + r:hbase + r + 128], tri)
                    pending = (p, E, roffs)
                    fill()
                attv(*pending)
                # batched normalize for this head pair
                nc.scalar.copy(dsb2[0:1, :], pn[0][64:65, :])
                nc.scalar.copy(dsb2[32:33, :], pn[1][64:65, :])
                bcp = psum.tile([128, 512], F32, tag="sh", name="bcp")
                nc.tensor.matmul(bcp, Msel, dsb2, start=True, stop=True)
                rc = np_.tile([128, 512], F32, tag="rc", name="rc")
                nc.vector.reciprocal(rc, bcp)
                for hh in range(2):
                    rs = slice(64 * hh, 64 * (hh + 1))
                    nc.vector.tensor_mul(attT[t][rs, qs], pn[hh][0:64, :],
                                         rc[rs, :])
                while emitted < len(fillers):
                    fillers[emitted]()
                    emitted += 1

        def wo_half_a(j, st):
            stg = 4 * j + st
            ss = slice(128 * stg, 128 * (stg + 1))
            oba = oap.tile([128, 1024], F32, tag="oba", name="oba")
            for ot in range(2):
                os_ = slice(512 * ot, 512 * (ot + 1))
                pw = psum.tile([128, 512], F32, tag="sh", name="pwa")
                nc.tensor.matmul(pw, attT[0][:, ss], wo[:, 512 * ot:512 * (ot + 1)],
                                 start=True, stop=True)
                nc.vector.tensor_copy(oba[:, os_], pw)
            return oba

        def wo_half_b(j, st, oba):
            stg = 4 * j + st
            ss = slice(128 * stg, 128 * (stg + 1))
            ob = op.tile([128, 1024], BF16, tag="ob", name="ob")
            pw = psum.tile([128, 1024], F32, tag="score", name="pwb")
            for ot in range(2):
                os_ = slice(512 * ot, 512 * (ot + 1))
                nc.tensor.matmul(pw[:, os_], attT[1][:, ss],
                                 wo[:, 1024 + 512 * ot:1024 + 512 * (ot + 1)],
                                 start=True, stop=True)
            nc.vector.tensor_add(ob, pw, oba)
            nc.sync.dma_start(out=out_d[ss, :], in_=ob)

        ilv = opts["ilv"]
        obas = {}

        def proj_fillers(sb):
            fl = []
            for (w, dest) in ((wq, QT), (wk, KT)):
                for t in range(2):
                    box = {}
                    fl.append(lambda sb=sb, w=w, t=t, box=box:
                              box.__setitem__("raw", qkproj_a(sb, w, t)))
                    fl.append(lambda sb=sb, dest=dest, t=t, box=box:
                              qkproj_b(sb, dest, t, box["raw"]))
            for st4 in range(4):
                fl.append(lambda st=4 * sb + st4: vproj(st))
            return fl

        for i in range(NSB + 2):
            fill0, fill1 = [], []
            if i < NSB:
                fl = proj_fillers(i)
                fill0, fill1 = fl[:8], fl[8:]
            if i == NSB:
                # last block: PE is starved while ACT grinds exp, so all the
                # deferred Wo work plus the attT[0]-half of Wo(j3) goes here
                fill0 = [lambda j=j, st=st: wo_chunk(j, st)
                         for j in range(2) for st in range(4)]
                fill1 = ([lambda st=st: wo_chunk(2, st) for st in range(4)] +
                         [lambda st=st: obas.__setitem__(st, wo_half_a(NSB - 1, st))
                          for st in range(4)])
            if i == NSB + 1:
                for st in range(4):
                    wo_half_b(NSB - 1, st, obas[st])
                continue
            if 1 <= i <= NSB and ilv:
                att_block(i - 1, fill0, fill1)
            else:
                for f in fill0 + fill1:
                    f()
                if 1 <= i <= NSB:
                    att_block(i - 1, [], [])


def get_program(use_rs=False, reps=1, **kw):
    opts = dict(DEFAULT_OPTS)
    opts.update(reps=reps, **kw)
    key = tuple(sorted(opts.items()))
    if key not in _CACHED:
        _CACHED[key] = _build_program(opts)
    return _CACHED[key]


def make_in_maps(x, Wq, Wk, Wv, Wo, token_positions):
    """Host-side sharding: per-core input dicts."""
    import ml_dtypes
    bf16 = ml_dtypes.bfloat16
    x = np.asarray(x, dtype=np.float32)
    Wq = np.asarray(Wq, dtype=np.float32)
    Wk = np.asarray(Wk, dtype=np.float32)
    Wv = np.asarray(Wv, dtype=np.float32)
    Wo = np.asarray(Wo, dtype=np.float32)
    pos = np.asarray(token_positions).astype(np.float32)

    # rope tables, feature-major [128, S]: row p -> pair index i = p % 32,
    # rows [0:32]=evens, [32:64]=odds per 64-row head block.
    i = np.arange(DH // 2, dtype=np.float32)
    d = THETA ** (2.0 * i / DH)                       # [32]
    tt = pos[None, :] / d[:, None]                    # [32, S]
    sin, cos = np.sin(tt), np.cos(tt)
    A = np.tile(cos, (4, 1)).astype(bf16)             # [128, S]
    # dest = raw*A + swap(raw)*B ; B: evens row -> -sin, odds row -> +sin
    Bt = np.tile(np.concatenate([-sin, sin], axis=0), (2, 1)).astype(bf16)

    # causal triangle mask [128, 128]: allow j >= p
    p = np.arange(128)[:, None]
    jj = np.arange(128)[None, :]
    tri = (jj >= p).astype(bf16)

    # PE swap permutation: out row i = raw row (i ^ 32)
    Pm = np.zeros((128, 128), np.float32)
    Pm[np.arange(128) ^ 32, np.arange(128)] = 1.0

    # normalize broadcast selector: den rows at partitions 0 and 32
    Msel = np.zeros((33, 128), bf16)
    Msel[0, 0:64] = 1.0
    Msel[32, 64:128] = 1.0

    # per-head Q/K row permutation: evens then odds
    i2 = np.arange(DH // 2)
    perm = np.concatenate(
        [np.concatenate([64 * h + 2 * i2, 64 * h + 2 * i2 + 1]) for h in range(H)])

    def pack8(wT):  # [1024, 256] -> [128, 2048] with k chunks side by side
        return np.concatenate([wT[128 * k:128 * (k + 1), :] for k in range(NKT)],
                              axis=1)

    def pack8t(wT):  # [1024, 256] -> [128, 2048], t-major: [t][k][128]
        return np.concatenate(
            [wT[128 * k:128 * (k + 1), 128 * t:128 * (t + 1)]
             for t in range(2) for k in range(NKT)], axis=1)

    def pack2(wT):  # [256, 1024] -> [128, 2048]
        return np.concatenate([wT[128 * t:128 * (t + 1), :] for t in range(2)],
                              axis=1)

    in_maps = []
    for c in range(CORES):
        b, g = c // 4, c % 4
        rows = perm[F * g:F * (g + 1)]
        nat = np.arange(F * g, F * (g + 1))
        in_maps.append({
            "xT": np.ascontiguousarray(x[b].T).astype(bf16),
            "wqP": np.ascontiguousarray(pack8t(Wq[rows, :].T)).astype(bf16),
            "wkP": np.ascontiguousarray(pack8t(Wk[rows, :].T)).astype(bf16),
            "wvP": np.ascontiguousarray(pack8(Wv[nat, :].T)).astype(bf16),
            "woP": np.ascontiguousarray(pack2(Wo[:, nat].T).astype(bf16)),
            "ropeA": A,
            "ropeB": Bt,
            "tri": tri,
            "Pm": Pm,
            "Msel": Msel,
        })
    return in_maps


def assemble(results, use_rs=False):
    out = np.empty((B, S, D), dtype=np.float32)
    for b in range(B):
        acc = results[4 * b]["partial"].astype(np.float32).copy()
        for g in range(1, 4):
            acc += results[4 * b + g]["partial"]
        out[b] = acc
    return out


def kernel(x, Wq, Wk, Wv, Wo, token_positions):
    from concourse.bass_utils import run_bass_kernel_spmd
    nc = get_program(False)
    in_maps = make_in_maps(x, Wq, Wk, Wv, Wo, token_positions)
    res = run_bass_kernel_spmd(nc, in_maps, list(range(CORES)))
    return assemble(res.results)
